# revision 14
# baseline (speedup 1.0000x reference)
"""Trainium2 Bass kernel for nn_BatchedSemiAttention (GNN message passing).

Math: the attention logit w[e,l] depends only on the SOURCE node col[e]:
    kq[g,l] = sum_d K*Q + x.(Wk bq + Wq bk)      (bk.bq const cancels in softmax)
    u[g,l]  = exp(kq[g,l])                       (|kq| small; no segment-max needed)
    U[g,l,:] = u[g,l] * V[g,l,:]
so the edge phase is a pure gather + segment-sum:
    agg[g,l] = (sum_{e in seg(g)} U[col[e],l]) / (sum_e u[col[e],l]) + bv
then SiLU + LayerNorm.

Sharding: row_indices sorted -> dest nodes partitioned into 8 ranges of 1280
(G padded to 10240); no collectives. Each core replicates the node-table
phase (bf16 [512 U | 4 u | 124 pad] records, 1280B rows for the dma_gather
256B-granularity rule).

The gather is Q7-descriptor-generation bound (~8 ns/edge), so the node table
is split in two DRAM tensors (sources 0..5119 / 5120..10239): each dest
block's edges (sorted by source) split into an A part gathered as soon as
the first table half is written -- overlapping the second half of phase 1 --
and a B part gathered after phase 1. A-part partial segment sums are spilled
from PSUM to SBUF and combined in the B-part epilogue. Segment-sums use
host-precomputed bf16 one-hot matrices as the stationary matmul operand.
"""

import sys
import numpy as np

if "/opt/trn_rl_repo" not in sys.path:
    sys.path.insert(0, "/opt/trn_rl_repo")

L, G = 4, 10000
INP, KEY, VAL = 128, 64, 128
LN_EPS = 1e-5
NCORES = 8
GPAD = 10240
DG = GPAD // NCORES          # 1280 destinations per core
NB = DG // 128               # 10 dest-blocks of 128 per core
NT = GPAD // 128             # 80 node tiles (phase 1)
NTH = NT // 2                # tiles per table half
GH = NTH * 128               # 5120 sources per table half
REC = 640                    # record bf16 elems: [512 U | 4 u | 124 pad] = 1280B

TRACE = False                # set by test harness for profiling runs
LAST_RESULT = {}             # exec_time etc. stashed here for the harness

_prog_cache = {}


def _build_program(TBP, tbs):
    """tbs[b][part] = chunk count (of 128 edges) for block b, table part."""
    import concourse.bass as bass
    import concourse.bacc as bacc
    import concourse.mybir as mybir
    import concourse.tile as tile
    from concourse.bass import broadcast_tensor_aps

    f32 = mybir.dt.float32
    bf16 = mybir.dt.bfloat16
    AX = mybir.AxisListType
    AL = mybir.AluOpType
    AF = mybir.ActivationFunctionType

    nc = bacc.Bacc()
    xTT = nc.dram_tensor("xTT", [INP, NT, L, 128], bf16, kind="ExternalInput")
    wcat = nc.dram_tensor("wcat", [INP, 256], bf16, kind="ExternalInput")
    v_in = nc.dram_tensor("v_in", [INP, 1], bf16, kind="ExternalInput")
    bv4 = nc.dram_tensor("bv4", [128, L * VAL], f32, kind="ExternalInput")
    gamma4 = nc.dram_tensor("gamma4", [128, L * VAL], f32, kind="ExternalInput")
    beta4 = nc.dram_tensor("beta4", [128, L * VAL], f32, kind="ExternalInput")
    eidx = nc.dram_tensor("eidx", [NB, 2, 128, TBP * 8], mybir.dt.int16,
                          kind="ExternalInput")
    ohd = nc.dram_tensor("ohd", [NB, 2, 128, TBP * 128], bf16,
                         kind="ExternalInput")
    out_d = nc.dram_tensor("out", [DG, L * VAL], f32, kind="ExternalOutput")
    tabA = nc.dram_tensor("tabA", [GH, REC], bf16)
    tabB = nc.dram_tensor("tabB", [GH, REC], bf16)

    with tile.TileContext(nc) as tc:
        with (
            tc.tile_pool(name="const", bufs=1) as constp,
            tc.tile_pool(name="xin", bufs=3) as xinp,
            tc.tile_pool(name="pskq", bufs=2, space="PSUM") as pskqp,
            tc.tile_pool(name="psv", bufs=2, space="PSUM") as psvp,
            tc.tile_pool(name="ps4", bufs=2, space="PSUM") as ps4p,
            tc.tile_pool(name="psU", bufs=2, space="PSUM") as psUp,
            tc.tile_pool(name="work", bufs=3) as workp,
            tc.tile_pool(name="rec", bufs=3) as recp,
            tc.tile_pool(name="gat", bufs=8) as gatp,
            tc.tile_pool(name="oh", bufs=3) as ohp,
            tc.tile_pool(name="fin", bufs=2) as finp,
        ):
            wcat_sb = constp.tile([INP, 256], bf16)
            nc.sync.dma_start(wcat_sb[:, :], wcat[:, :])
            v_sb = constp.tile([INP, 1], bf16)
            nc.sync.dma_start(v_sb[:, :], v_in[:, :])
            bv_sb = constp.tile([128, L * VAL], f32)
            nc.sync.dma_start(bv_sb[:, :], bv4[:, :])
            gam_sb = constp.tile([128, L * VAL], f32)
            nc.sync.dma_start(gam_sb[:, :], gamma4[:, :])
            bet_sb = constp.tile([128, L * VAL], f32)
            nc.sync.dma_start(bet_sb[:, :], beta4[:, :])
            accU = constp.tile([128, NB, 512], f32)
            accS = constp.tile([128, NB, L], f32)

            # ---- phase 1: node table (projections, u, U) -------------------
            for i in range(NT):
                tab = tabA if i < NTH else tabB
                r0 = (i % NTH) * 128
                xt4 = xinp.tile([128, L, 128], bf16, tag="xt4")
                # issue from the Scalar sequencer: the Sync sequencer's
                # ~600ns/DMA issue cost plus its head-of-line wait on the
                # table write would serialize phase 1
                nc.scalar.dma_start(xt4[:, :, :], xTT[:, i, :, :])
                pskq = pskqp.tile([128, L, 128], f32, tag="pskq")
                psv = psvp.tile([128, L, 128], f32, tag="psv")
                psvx = ps4p.tile([128, L], f32, tag="ps4")
                for l in range(L):
                    nc.tensor.matmul(pskq[:, l, :], xt4[:, l, :],
                                     wcat_sb[:, 0:128], start=True, stop=True)
                    nc.tensor.matmul(psvx[:, l:l + 1], xt4[:, l, :], v_sb[:, :],
                                     start=True, stop=True)
                for l in range(L):
                    nc.tensor.matmul(psv[:, l, :], xt4[:, l, :],
                                     wcat_sb[:, 128:256], start=True, stop=True)
                rec = recp.tile([128, REC], bf16, tag="rec")
                qs = workp.tile([128, L, KEY], f32, tag="qs")
                nc.scalar.activation(qs[:, :, :], pskq[:, :, 64:128], AF.Copy)
                scr = workp.tile([128, L, KEY], f32, tag="scr")
                nc.vector.tensor_tensor(scr[:, :, :], pskq[:, :, 0:64],
                                        qs[:, :, :], AL.mult)
                kq = workp.tile([128, L], f32, tag="kq")
                nc.vector.tensor_reduce(kq[:, :], scr[:, :, :], AX.X, AL.add)
                kq2 = workp.tile([128, L], f32, tag="kq2")
                nc.vector.tensor_tensor(kq2[:, :], kq[:, :], psvx[:, :], AL.add)
                nc.scalar.activation(rec[:, 512:516], kq2[:, :], AF.Exp)
                # U_l = V_l * u_l for all 4 l in one DVE op (stride-0 bcast u)
                u4 = rec[:, 512:516].rearrange("p (l o) -> p l o", o=1)
                rec4 = rec[:, 0:512].rearrange("p (l v) -> p l v", l=L)
                a, b = broadcast_tensor_aps(psv[:, :, :], u4)
                nc.vector.tensor_tensor(rec4, a, b, AL.mult)
                nc.sync.dma_start(tab[r0:r0 + 128, :], rec[:, :])

            # ---- phase 2a: A-part gathers + partial segment sums -----------
            for bb in range(NB):
                tba = tbs[bb][0]
                idx_sb = ohp.tile([128, TBP * 8], mybir.dt.int16, tag="idx")
                nc.gpsimd.dma_start(idx_sb[:, 0:tba * 8],
                                    eidx[bb, 0, :, 0:tba * 8])
                oh_sb = ohp.tile([128, TBP * 128], bf16, tag="oh")
                nc.gpsimd.dma_start(oh_sb[:, 0:tba * 128],
                                    ohd[bb, 0, :, 0:tba * 128])
                gt = gatp.tile([128, TBP, REC], bf16, tag="gt")
                nc.gpsimd.dma_gather(gt[:, 0:tba, :], tabA[:, :],
                                     idx_sb[:, 0:tba * 8], tba * 128, tba * 128,
                                     REC, elem_step=REC, single_packet=False)
                psU = psUp.tile([128, 512], f32, tag="psU")
                psS = ps4p.tile([128, L], f32, tag="ps4")
                for t in range(tba):
                    oh_t = oh_sb[:, t * 128:(t + 1) * 128]
                    nc.tensor.matmul(psU[:, :], oh_t, gt[:, t, 0:512],
                                     start=(t == 0), stop=(t == tba - 1))
                    nc.tensor.matmul(psS[:, :], oh_t, gt[:, t, 512:516],
                                     start=(t == 0), stop=(t == tba - 1))
                nc.scalar.activation(accU[:, bb, :], psU[:, :], AF.Copy)
                nc.scalar.activation(accS[:, bb, :], psS[:, :], AF.Copy)

            # ---- phase 2b: B-part gathers + combine + epilogue -------------
            for bb in range(NB):
                tbb = tbs[bb][1]
                idx_sb = ohp.tile([128, TBP * 8], mybir.dt.int16, tag="idx")
                nc.gpsimd.dma_start(idx_sb[:, 0:tbb * 8],
                                    eidx[bb, 1, :, 0:tbb * 8])
                oh_sb = ohp.tile([128, TBP * 128], bf16, tag="oh")
                nc.gpsimd.dma_start(oh_sb[:, 0:tbb * 128],
                                    ohd[bb, 1, :, 0:tbb * 128])
                gt = gatp.tile([128, TBP, REC], bf16, tag="gt")
                nc.gpsimd.dma_gather(gt[:, 0:tbb, :], tabB[:, :],
                                     idx_sb[:, 0:tbb * 8], tbb * 128, tbb * 128,
                                     REC, elem_step=REC, single_packet=False)
                psU = psUp.tile([128, 512], f32, tag="psU")
                psS = ps4p.tile([128, L], f32, tag="ps4")
                for t in range(tbb):
                    oh_t = oh_sb[:, t * 128:(t + 1) * 128]
                    nc.tensor.matmul(psU[:, :], oh_t, gt[:, t, 0:512],
                                     start=(t == 0), stop=(t == tbb - 1))
                    nc.tensor.matmul(psS[:, :], oh_t, gt[:, t, 512:516],
                                     start=(t == 0), stop=(t == tbb - 1))
                totU = finp.tile([128, 512], f32, tag="totU")
                nc.vector.tensor_tensor(totU[:, :], psU[:, :], accU[:, bb, :],
                                        AL.add)
                totS = finp.tile([128, L], f32, tag="totS")
                nc.vector.tensor_tensor(totS[:, :], psS[:, :], accS[:, bb, :],
                                        AL.add)
                s_sb = finp.tile([128, L], f32, tag="s")
                nc.vector.tensor_scalar(s_sb[:, :], totS[:, :], 1e-30, None,
                                        AL.max)
                rcp = finp.tile([128, L], f32, tag="rcp")
                nc.vector.reciprocal(rcp[:, :], s_sb[:, :])
                bv_ap = bv_sb[:, :].rearrange("p (l v) -> p l v", l=L)
                tot4 = totU[:, :].rearrange("p (l v) -> p l v", l=L)
                sc = finp.tile([128, L, VAL], f32, tag="sc")
                for l in range(L):
                    # sc_l = totU_l * rcp_l + bv_l  (fused)
                    nc.vector.scalar_tensor_tensor(
                        sc[:, l, :], tot4[:, l, :],
                        rcp[:, l:l + 1], bv_ap[:, l, :], AL.mult, AL.add)
                sil = finp.tile([128, L, VAL], f32, tag="sil")
                nc.scalar.activation(sil[:, :, :], sc[:, :, :], AF.Silu)
                mur = finp.tile([128, L], f32, tag="mur")
                nc.vector.tensor_reduce(mur[:, :], sil[:, :, :], AX.X, AL.add)
                mu = finp.tile([128, L], f32, tag="mu")
                nc.vector.tensor_scalar(mu[:, :], mur[:, :], 1.0 / VAL, None,
                                        AL.mult)
                sq = finp.tile([128, L, VAL], f32, tag="sq")
                nc.vector.tensor_tensor(sq[:, :, :], sil[:, :, :],
                                        sil[:, :, :], AL.mult)
                ssq = finp.tile([128, L], f32, tag="ssq")
                nc.vector.tensor_reduce(ssq[:, :], sq[:, :, :], AX.X, AL.add)
                var = finp.tile([128, L], f32, tag="var")
                nc.vector.tensor_scalar(var[:, :], ssq[:, :], 1.0 / VAL, LN_EPS,
                                        AL.mult, AL.add)
                musq = finp.tile([128, L], f32, tag="musq")
                nc.vector.tensor_tensor(musq[:, :], mu[:, :], mu[:, :], AL.mult)
                var2 = finp.tile([128, L], f32, tag="var2")
                nc.vector.tensor_tensor(var2[:, :], var[:, :], musq[:, :],
                                        AL.subtract)
                std = finp.tile([128, L], f32, tag="std")
                nc.scalar.activation(std[:, :], var2[:, :], AF.Sqrt)
                rstd = finp.tile([128, L], f32, tag="rstd")
                nc.vector.reciprocal(rstd[:, :], std[:, :])
                osb = finp.tile([128, L, VAL], f32, tag="osb")
                for l in range(L):
                    nc.vector.tensor_scalar(osb[:, l, :], sil[:, l, :],
                                            mu[:, l:l + 1], rstd[:, l:l + 1],
                                            AL.subtract, AL.mult)
                gam_ap = gam_sb[:, :].rearrange("p (l v) -> p l v", l=L)
                bet_ap = bet_sb[:, :].rearrange("p (l v) -> p l v", l=L)
                nc.vector.tensor_tensor(osb[:, :, :], osb[:, :, :], gam_ap,
                                        AL.mult)
                nc.vector.tensor_tensor(osb[:, :, :], osb[:, :, :], bet_ap,
                                        AL.add)
                nc.sync.dma_start(out_d[bb * 128:(bb + 1) * 128, :],
                                  osb[:, :, :])
    nc.compile()
    return nc


def _prepare(x, Wk, bk, Wq, bq, Wv, bv, gamma, beta, row_indices, col_indices):
    import ml_dtypes
    bf16 = ml_dtypes.bfloat16

    x = np.asarray(x, dtype=np.float32)
    Wk = np.asarray(Wk, dtype=np.float32)
    bk = np.asarray(bk, dtype=np.float32)
    Wq = np.asarray(Wq, dtype=np.float32)
    bq = np.asarray(bq, dtype=np.float32)
    Wv = np.asarray(Wv, dtype=np.float32)
    bv = np.asarray(bv, dtype=np.float32)
    gamma = np.asarray(gamma, dtype=np.float32)
    beta = np.asarray(beta, dtype=np.float32)
    row = np.asarray(row_indices).astype(np.int64)
    col = np.asarray(col_indices).astype(np.int64)

    if row.size and np.any(np.diff(row) < 0):
        o = np.argsort(row, kind="stable")
        row, col = row[o], col[o]

    # host-side index prep: per 128-dest block, edges split by source half
    bounds = np.searchsorted(row, np.arange(0, GPAD + 1, 128))
    tbs = []      # [core*NB + b] -> (tba, tbb)
    parts = []    # per (core-block, part): (cols_local, rows_local)
    for k in range(NCORES * NB):
        lo, hi = bounds[k], bounds[k + 1]
        cb, rb = col[lo:hi], row[lo:hi] - (k * 128)
        pa, pb = cb < GH, cb >= GH
        entry = []
        for m, base in ((pa, 0), (pb, GH)):
            cs, rs = cb[m], rb[m]
            oo = np.argsort(cs, kind="stable")   # ascending source addresses
            entry.append((cs[oo] - base, rs[oo]))
        parts.append(entry)
        tbs.append(tuple(max(1, int(np.ceil(len(e[0]) / 128.0)))
                         for e in entry))
    TBP = max(max(t) for t in tbs)

    eidx = np.zeros((NCORES, NB, 2, 128, TBP * 8), np.int16)
    ohd = np.zeros((NCORES, NB, 2, 128, TBP * 128), bf16)
    for c in range(NCORES):
        for b in range(NB):
            for p in range(2):
                cs, rs = parts[c * NB + b][p]
                n = len(cs)
                tbn = tbs[c * NB + b][p]
                eb = tbn * 128
                cbuf = np.zeros(eb, np.int64)
                cbuf[:n] = cs
                # idxs wrapped in 16 partitions, replicated across 8 Q7 cores
                eidx[c, b, p, :, 0:tbn * 8] = np.tile(
                    cbuf.reshape(eb // 16, 16).T.astype(np.int16), (8, 1))
                oh = np.zeros((eb, 128), np.float32)
                oh[np.arange(n), rs] = 1.0
                ohd[c, b, p, :, 0:tbn * 128] = oh.reshape(
                    tbn, 128, 128).transpose(1, 0, 2).reshape(
                    128, tbn * 128).astype(bf16)

    xp = np.zeros((L, GPAD, INP), np.float32)
    xp[:, :G] = x
    # xTT[d, i, l, g'] = x[l, i*128+g', d]: 1KB contiguous lines per partition
    xTT = np.ascontiguousarray(
        xp.transpose(2, 1, 0).reshape(INP, NT, 128, L).transpose(0, 1, 3, 2)
    ).astype(bf16)
    wcat = np.ascontiguousarray(
        np.concatenate([Wk, Wq, Wv], axis=1)).astype(bf16)
    v_host = (Wk @ bq + Wq @ bk).astype(bf16)[:, None]
    bv4h = np.ascontiguousarray(
        np.broadcast_to(np.tile(bv, L)[None, :], (128, L * VAL)))
    gamma4 = np.ascontiguousarray(
        np.broadcast_to(np.tile(gamma, L)[None, :], (128, L * VAL)))
    beta4 = np.ascontiguousarray(
        np.broadcast_to(np.tile(beta, L)[None, :], (128, L * VAL)))

    in_maps = []
    for c in range(NCORES):
        in_maps.append({
            "xTT": xTT, "wcat": wcat, "v_in": v_host, "bv4": bv4h,
            "gamma4": gamma4, "beta4": beta4,
            "eidx": np.ascontiguousarray(eidx[c]),
            "ohd": np.ascontiguousarray(ohd[c]),
        })
    # per-core chunk counts differ; build per-core programs keyed by counts
    tbs_by_core = [tuple(tbs[c * NB:(c + 1) * NB]) for c in range(NCORES)]
    return TBP, tbs_by_core, in_maps


def kernel(x, Wk, bk, Wq, bq, Wv, bv, gamma, beta, row_indices, col_indices):
    from concourse.bass_utils import run_bass_kernel_spmd

    TBP, tbs_by_core, in_maps = _prepare(x, Wk, bk, Wq, bq, Wv, bv, gamma,
                                         beta, row_indices, col_indices)
    # SPMD: one program for all cores -> pad every block to the max chunk
    # count across cores for that (block, part)? No -- the program must be
    # identical across cores, so use the per-(b, part) max.
    tbs_max = tuple(
        tuple(max(tbs_by_core[c][b][p] for c in range(NCORES))
              for p in range(2))
        for b in range(NB)
    )
    key = (TBP, tbs_max)
    if key not in _prog_cache:
        _prog_cache.clear()
        _prog_cache[key] = _build_program(TBP, tbs_max)
    nc = _prog_cache[key]

    res = run_bass_kernel_spmd(nc, in_maps, core_ids=list(range(NCORES)),
                               trace=TRACE)
    LAST_RESULT["exec_time_ns"] = getattr(res, "exec_time_ns", None)

    full = np.concatenate([res.results[c]["out"] for c in range(NCORES)], axis=0)
    out = np.ascontiguousarray(
        full[:G].reshape(G, L, VAL).transpose(1, 0, 2)).astype(np.float32)
    return out


# revision 16
# speedup vs baseline: 1.1432x; 1.1432x over previous
"""Trainium2 Bass kernel for nn_BatchedSemiAttention (GNN message passing).

Math: the attention logit w[e,l] depends only on the SOURCE node col[e]:
    kq[g,l] = sum_d K*Q + x.(Wk bq + Wq bk)      (bk.bq const cancels in softmax)
    u[g,l]  = exp(kq[g,l])                       (|kq| small; no segment-max needed)
    U[g,l,:] = u[g,l] * V[g,l,:]
so the edge phase is a pure gather + segment-sum:
    agg[g,l] = (sum_{e in seg(g)} U[col[e],l]) / (sum_e u[col[e],l]) + bv
then SiLU + LayerNorm.

Sharding: row_indices sorted -> dest nodes partitioned into 8 ranges of 1280
(G padded to 10240); no collectives. Each core replicates the node-table
phase (bf16 [512 U | 4 u | 124 pad] records, 1280B rows for the dma_gather
256B-granularity rule).

The gather is Q7-descriptor-generation bound (~8 ns/edge), so the node table
is split in two DRAM tensors (sources 0..5119 / 5120..10239): each dest
block's edges (sorted by source) split into an A part gathered as soon as
the first table half is written -- overlapping the second half of phase 1 --
and a B part gathered after phase 1. A-part partial segment sums are spilled
from PSUM to SBUF and combined in the B-part epilogue. Segment-sums use
host-precomputed bf16 one-hot matrices as the stationary matmul operand.
"""

import sys
import numpy as np

if "/opt/trn_rl_repo" not in sys.path:
    sys.path.insert(0, "/opt/trn_rl_repo")

L, G = 4, 10000
INP, KEY, VAL = 128, 64, 128
LN_EPS = 1e-5
NCORES = 8
GPAD = 10240
DG = GPAD // NCORES          # 1280 destinations per core
NB = DG // 128               # 10 dest-blocks of 128 per core
NT = GPAD // 128             # 80 node tiles (phase 1)
NTH = NT // 2                # tiles per table half
GH = NTH * 128               # 5120 sources per table half
REC = 640                    # record bf16 elems: [512 U | 4 u | 124 pad] = 1280B

TRACE = False                # set by test harness for profiling runs
LAST_RESULT = {}             # exec_time etc. stashed here for the harness

_prog_cache = {}


def _build_program(TBP, tbs):
    """tbs[b][part] = chunk count (of 128 edges) for block b, table part."""
    import concourse.bass as bass
    import concourse.bacc as bacc
    import concourse.mybir as mybir
    import concourse.tile as tile
    from concourse.bass import broadcast_tensor_aps

    f32 = mybir.dt.float32
    bf16 = mybir.dt.bfloat16
    AX = mybir.AxisListType
    AL = mybir.AluOpType
    AF = mybir.ActivationFunctionType

    nc = bacc.Bacc()
    xTT = nc.dram_tensor("xTT", [INP, NT, L, 128], bf16, kind="ExternalInput")
    wcat = nc.dram_tensor("wcat", [INP, 256], bf16, kind="ExternalInput")
    v_in = nc.dram_tensor("v_in", [INP, 1], bf16, kind="ExternalInput")
    bv4 = nc.dram_tensor("bv4", [128, L * VAL], f32, kind="ExternalInput")
    gamma4 = nc.dram_tensor("gamma4", [128, L * VAL], f32, kind="ExternalInput")
    beta4 = nc.dram_tensor("beta4", [128, L * VAL], f32, kind="ExternalInput")
    eidx = nc.dram_tensor("eidx", [NB, 2, 128, TBP * 8], mybir.dt.int16,
                          kind="ExternalInput")
    ohd = nc.dram_tensor("ohd", [NB, 2, 128, TBP * 128], bf16,
                         kind="ExternalInput")
    out_d = nc.dram_tensor("out", [DG, L * VAL], f32, kind="ExternalOutput")
    tabA = nc.dram_tensor("tabA", [GH, REC], bf16)
    tabB = nc.dram_tensor("tabB", [GH, REC], bf16)

    with tile.TileContext(nc) as tc:
        with (
            tc.tile_pool(name="const", bufs=1) as constp,
            tc.tile_pool(name="xin", bufs=7) as xinp,
            tc.tile_pool(name="pskq", bufs=2, space="PSUM") as pskqp,
            tc.tile_pool(name="psv", bufs=2, space="PSUM") as psvp,
            tc.tile_pool(name="ps4", bufs=2, space="PSUM") as ps4p,
            tc.tile_pool(name="psU", bufs=2, space="PSUM") as psUp,
            tc.tile_pool(name="work", bufs=3) as workp,
            tc.tile_pool(name="rec", bufs=3) as recp,
            tc.tile_pool(name="gat", bufs=8) as gatp,
            tc.tile_pool(name="oh", bufs=3) as ohp,
            tc.tile_pool(name="fin", bufs=2) as finp,
        ):
            wcat_sb = constp.tile([INP, 256], bf16)
            nc.sync.dma_start(wcat_sb[:, :], wcat[:, :])
            v_sb = constp.tile([INP, 1], bf16)
            nc.sync.dma_start(v_sb[:, :], v_in[:, :])
            bv_sb = constp.tile([128, L * VAL], f32)
            nc.sync.dma_start(bv_sb[:, :], bv4[:, :])
            gam_sb = constp.tile([128, L * VAL], f32)
            nc.sync.dma_start(gam_sb[:, :], gamma4[:, :])
            bet_sb = constp.tile([128, L * VAL], f32)
            nc.sync.dma_start(bet_sb[:, :], beta4[:, :])
            accU = constp.tile([128, NB, 512], f32)
            accS = constp.tile([128, NB, L], f32)

            # ---- phase 1: node table (projections, u, U) -------------------
            # software-pipelined xt4 prefetch: issue loads PF tiles ahead so
            # the sync sequencer's head-of-line wait on table writes never
            # delays them
            PF = 6
            xts = {}
            for i in range(PF):
                xts[i] = xinp.tile([128, L, 128], bf16, tag="xt4", name="xt4")
                nc.sync.dma_start(xts[i][:, :, :], xTT[:, i, :, :])
            for i in range(NT):
                tab = tabA if i < NTH else tabB
                r0 = (i % NTH) * 128
                if i + PF < NT:
                    xts[i + PF] = xinp.tile([128, L, 128], bf16, tag="xt4",
                                            name="xt4")
                    nc.sync.dma_start(xts[i + PF][:, :, :], xTT[:, i + PF, :, :])
                xt4 = xts.pop(i)
                pskq = pskqp.tile([128, L, 128], f32, tag="pskq")
                psv = psvp.tile([128, L, 128], f32, tag="psv")
                psvx = ps4p.tile([128, L], f32, tag="ps4")
                # per l: kq, vx, v share the same stationary operand ->
                # walrus elides the redundant LDWEIGHTS
                for l in range(L):
                    nc.tensor.matmul(pskq[:, l, :], xt4[:, l, :],
                                     wcat_sb[:, 0:128], start=True, stop=True)
                    nc.tensor.matmul(psvx[:, l:l + 1], xt4[:, l, :], v_sb[:, :],
                                     start=True, stop=True)
                    nc.tensor.matmul(psv[:, l, :], xt4[:, l, :],
                                     wcat_sb[:, 128:256], start=True, stop=True)
                rec = recp.tile([128, REC], bf16, tag="rec")
                qs = workp.tile([128, L, KEY], f32, tag="qs")
                nc.scalar.activation(qs[:, :, :], pskq[:, :, 64:128], AF.Copy)
                scr = workp.tile([128, L, KEY], f32, tag="scr")
                nc.vector.tensor_tensor(scr[:, :, :], pskq[:, :, 0:64],
                                        qs[:, :, :], AL.mult)
                kq = workp.tile([128, L], f32, tag="kq")
                nc.vector.tensor_reduce(kq[:, :], scr[:, :, :], AX.X, AL.add)
                kq2 = workp.tile([128, L], f32, tag="kq2")
                nc.vector.tensor_tensor(kq2[:, :], kq[:, :], psvx[:, :], AL.add)
                nc.scalar.activation(rec[:, 512:516], kq2[:, :], AF.Exp)
                # U_l = V_l * u_l for all 4 l in one DVE op (stride-0 bcast u)
                u4 = rec[:, 512:516].rearrange("p (l o) -> p l o", o=1)
                rec4 = rec[:, 0:512].rearrange("p (l v) -> p l v", l=L)
                a, b = broadcast_tensor_aps(psv[:, :, :], u4)
                nc.vector.tensor_tensor(rec4, a, b, AL.mult)
                nc.sync.dma_start(tab[r0:r0 + 128, :], rec[:, :])

            # ---- phase 2a: A-part gathers + partial segment sums -----------
            for bb in range(NB):
                tba = tbs[bb][0]
                idx_sb = ohp.tile([128, TBP * 8], mybir.dt.int16, tag="idx")
                nc.gpsimd.dma_start(idx_sb[:, 0:tba * 8],
                                    eidx[bb, 0, :, 0:tba * 8])
                oh_sb = ohp.tile([128, TBP * 128], bf16, tag="oh")
                nc.gpsimd.dma_start(oh_sb[:, 0:tba * 128],
                                    ohd[bb, 0, :, 0:tba * 128])
                gt = gatp.tile([128, TBP, REC], bf16, tag="gt")
                nc.gpsimd.dma_gather(gt[:, 0:tba, :], tabA[:, :],
                                     idx_sb[:, 0:tba * 8], tba * 128, tba * 128,
                                     REC, elem_step=REC, single_packet=False)
                psU = psUp.tile([128, 512], f32, tag="psU")
                psS = ps4p.tile([128, L], f32, tag="ps4")
                for t in range(tba):
                    oh_t = oh_sb[:, t * 128:(t + 1) * 128]
                    nc.tensor.matmul(psU[:, :], oh_t, gt[:, t, 0:512],
                                     start=(t == 0), stop=(t == tba - 1))
                    nc.tensor.matmul(psS[:, :], oh_t, gt[:, t, 512:516],
                                     start=(t == 0), stop=(t == tba - 1))
                nc.scalar.activation(accU[:, bb, :], psU[:, :], AF.Copy)
                nc.scalar.activation(accS[:, bb, :], psS[:, :], AF.Copy)

            # ---- phase 2b: B-part gathers + combine + epilogue -------------
            for bb in range(NB):
                tbb = tbs[bb][1]
                idx_sb = ohp.tile([128, TBP * 8], mybir.dt.int16, tag="idx")
                nc.gpsimd.dma_start(idx_sb[:, 0:tbb * 8],
                                    eidx[bb, 1, :, 0:tbb * 8])
                oh_sb = ohp.tile([128, TBP * 128], bf16, tag="oh")
                nc.gpsimd.dma_start(oh_sb[:, 0:tbb * 128],
                                    ohd[bb, 1, :, 0:tbb * 128])
                gt = gatp.tile([128, TBP, REC], bf16, tag="gt")
                nc.gpsimd.dma_gather(gt[:, 0:tbb, :], tabB[:, :],
                                     idx_sb[:, 0:tbb * 8], tbb * 128, tbb * 128,
                                     REC, elem_step=REC, single_packet=False)
                psU = psUp.tile([128, 512], f32, tag="psU")
                psS = ps4p.tile([128, L], f32, tag="ps4")
                for t in range(tbb):
                    oh_t = oh_sb[:, t * 128:(t + 1) * 128]
                    nc.tensor.matmul(psU[:, :], oh_t, gt[:, t, 0:512],
                                     start=(t == 0), stop=(t == tbb - 1))
                    nc.tensor.matmul(psS[:, :], oh_t, gt[:, t, 512:516],
                                     start=(t == 0), stop=(t == tbb - 1))
                totU = finp.tile([128, 512], f32, tag="totU")
                nc.vector.tensor_tensor(totU[:, :], psU[:, :], accU[:, bb, :],
                                        AL.add)
                totS = finp.tile([128, L], f32, tag="totS")
                nc.vector.tensor_tensor(totS[:, :], psS[:, :], accS[:, bb, :],
                                        AL.add)
                s_sb = finp.tile([128, L], f32, tag="s")
                nc.vector.tensor_scalar(s_sb[:, :], totS[:, :], 1e-30, None,
                                        AL.max)
                rcp = finp.tile([128, L], f32, tag="rcp")
                nc.vector.reciprocal(rcp[:, :], s_sb[:, :])
                bv_ap = bv_sb[:, :].rearrange("p (l v) -> p l v", l=L)
                tot4 = totU[:, :].rearrange("p (l v) -> p l v", l=L)
                sc = finp.tile([128, L, VAL], f32, tag="sc")
                for l in range(L):
                    # sc_l = totU_l * rcp_l + bv_l  (fused)
                    nc.vector.scalar_tensor_tensor(
                        sc[:, l, :], tot4[:, l, :],
                        rcp[:, l:l + 1], bv_ap[:, l, :], AL.mult, AL.add)
                sil = finp.tile([128, L, VAL], f32, tag="sil")
                nc.scalar.activation(sil[:, :, :], sc[:, :, :], AF.Silu)
                mur = finp.tile([128, L], f32, tag="mur")
                nc.vector.tensor_reduce(mur[:, :], sil[:, :, :], AX.X, AL.add)
                mu = finp.tile([128, L], f32, tag="mu")
                nc.vector.tensor_scalar(mu[:, :], mur[:, :], 1.0 / VAL, None,
                                        AL.mult)
                sq = finp.tile([128, L, VAL], f32, tag="sq")
                nc.vector.tensor_tensor(sq[:, :, :], sil[:, :, :],
                                        sil[:, :, :], AL.mult)
                ssq = finp.tile([128, L], f32, tag="ssq")
                nc.vector.tensor_reduce(ssq[:, :], sq[:, :, :], AX.X, AL.add)
                var = finp.tile([128, L], f32, tag="var")
                nc.vector.tensor_scalar(var[:, :], ssq[:, :], 1.0 / VAL, LN_EPS,
                                        AL.mult, AL.add)
                musq = finp.tile([128, L], f32, tag="musq")
                nc.vector.tensor_tensor(musq[:, :], mu[:, :], mu[:, :], AL.mult)
                var2 = finp.tile([128, L], f32, tag="var2")
                nc.vector.tensor_tensor(var2[:, :], var[:, :], musq[:, :],
                                        AL.subtract)
                std = finp.tile([128, L], f32, tag="std")
                nc.scalar.activation(std[:, :], var2[:, :], AF.Sqrt)
                rstd = finp.tile([128, L], f32, tag="rstd")
                nc.vector.reciprocal(rstd[:, :], std[:, :])
                osb = finp.tile([128, L, VAL], f32, tag="osb")
                for l in range(L):
                    nc.vector.tensor_scalar(osb[:, l, :], sil[:, l, :],
                                            mu[:, l:l + 1], rstd[:, l:l + 1],
                                            AL.subtract, AL.mult)
                gam_ap = gam_sb[:, :].rearrange("p (l v) -> p l v", l=L)
                bet_ap = bet_sb[:, :].rearrange("p (l v) -> p l v", l=L)
                nc.vector.tensor_tensor(osb[:, :, :], osb[:, :, :], gam_ap,
                                        AL.mult)
                nc.vector.tensor_tensor(osb[:, :, :], osb[:, :, :], bet_ap,
                                        AL.add)
                nc.sync.dma_start(out_d[bb * 128:(bb + 1) * 128, :],
                                  osb[:, :, :])
    nc.compile()
    return nc


def _prepare(x, Wk, bk, Wq, bq, Wv, bv, gamma, beta, row_indices, col_indices):
    import ml_dtypes
    bf16 = ml_dtypes.bfloat16

    x = np.asarray(x, dtype=np.float32)
    Wk = np.asarray(Wk, dtype=np.float32)
    bk = np.asarray(bk, dtype=np.float32)
    Wq = np.asarray(Wq, dtype=np.float32)
    bq = np.asarray(bq, dtype=np.float32)
    Wv = np.asarray(Wv, dtype=np.float32)
    bv = np.asarray(bv, dtype=np.float32)
    gamma = np.asarray(gamma, dtype=np.float32)
    beta = np.asarray(beta, dtype=np.float32)
    row = np.asarray(row_indices).astype(np.int64)
    col = np.asarray(col_indices).astype(np.int64)

    if row.size and np.any(np.diff(row) < 0):
        o = np.argsort(row, kind="stable")
        row, col = row[o], col[o]

    # host-side index prep: per 128-dest block, edges split by source half
    bounds = np.searchsorted(row, np.arange(0, GPAD + 1, 128))
    tbs = []      # [core*NB + b] -> (tba, tbb)
    parts = []    # per (core-block, part): (cols_local, rows_local)
    for k in range(NCORES * NB):
        lo, hi = bounds[k], bounds[k + 1]
        cb, rb = col[lo:hi], row[lo:hi] - (k * 128)
        pa, pb = cb < GH, cb >= GH
        entry = []
        for m, base in ((pa, 0), (pb, GH)):
            cs, rs = cb[m], rb[m]
            oo = np.argsort(cs, kind="stable")   # ascending source addresses
            entry.append((cs[oo] - base, rs[oo]))
        parts.append(entry)
        tbs.append(tuple(max(1, int(np.ceil(len(e[0]) / 128.0)))
                         for e in entry))
    TBP = max(max(t) for t in tbs)

    eidx = np.zeros((NCORES, NB, 2, 128, TBP * 8), np.int16)
    ohd = np.zeros((NCORES, NB, 2, 128, TBP * 128), bf16)
    for c in range(NCORES):
        for b in range(NB):
            for p in range(2):
                cs, rs = parts[c * NB + b][p]
                n = len(cs)
                tbn = tbs[c * NB + b][p]
                eb = tbn * 128
                cbuf = np.zeros(eb, np.int64)
                cbuf[:n] = cs
                # idxs wrapped in 16 partitions, replicated across 8 Q7 cores
                eidx[c, b, p, :, 0:tbn * 8] = np.tile(
                    cbuf.reshape(eb // 16, 16).T.astype(np.int16), (8, 1))
                oh = np.zeros((eb, 128), np.float32)
                oh[np.arange(n), rs] = 1.0
                ohd[c, b, p, :, 0:tbn * 128] = oh.reshape(
                    tbn, 128, 128).transpose(1, 0, 2).reshape(
                    128, tbn * 128).astype(bf16)

    xp = np.zeros((L, GPAD, INP), np.float32)
    xp[:, :G] = x
    # xTT[d, i, l, g'] = x[l, i*128+g', d]: 1KB contiguous lines per partition
    xTT = np.ascontiguousarray(
        xp.transpose(2, 1, 0).reshape(INP, NT, 128, L).transpose(0, 1, 3, 2)
    ).astype(bf16)
    wcat = np.ascontiguousarray(
        np.concatenate([Wk, Wq, Wv], axis=1)).astype(bf16)
    v_host = (Wk @ bq + Wq @ bk).astype(bf16)[:, None]
    bv4h = np.ascontiguousarray(
        np.broadcast_to(np.tile(bv, L)[None, :], (128, L * VAL)))
    gamma4 = np.ascontiguousarray(
        np.broadcast_to(np.tile(gamma, L)[None, :], (128, L * VAL)))
    beta4 = np.ascontiguousarray(
        np.broadcast_to(np.tile(beta, L)[None, :], (128, L * VAL)))

    in_maps = []
    for c in range(NCORES):
        in_maps.append({
            "xTT": xTT, "wcat": wcat, "v_in": v_host, "bv4": bv4h,
            "gamma4": gamma4, "beta4": beta4,
            "eidx": np.ascontiguousarray(eidx[c]),
            "ohd": np.ascontiguousarray(ohd[c]),
        })
    # per-core chunk counts differ; build per-core programs keyed by counts
    tbs_by_core = [tuple(tbs[c * NB:(c + 1) * NB]) for c in range(NCORES)]
    return TBP, tbs_by_core, in_maps


def kernel(x, Wk, bk, Wq, bq, Wv, bv, gamma, beta, row_indices, col_indices):
    from concourse.bass_utils import run_bass_kernel_spmd

    TBP, tbs_by_core, in_maps = _prepare(x, Wk, bk, Wq, bq, Wv, bv, gamma,
                                         beta, row_indices, col_indices)
    # SPMD: one program for all cores -> pad every block to the max chunk
    # count across cores for that (block, part)? No -- the program must be
    # identical across cores, so use the per-(b, part) max.
    tbs_max = tuple(
        tuple(max(tbs_by_core[c][b][p] for c in range(NCORES))
              for p in range(2))
        for b in range(NB)
    )
    key = (TBP, tbs_max)
    if key not in _prog_cache:
        _prog_cache.clear()
        _prog_cache[key] = _build_program(TBP, tbs_max)
    nc = _prog_cache[key]

    res = run_bass_kernel_spmd(nc, in_maps, core_ids=list(range(NCORES)),
                               trace=TRACE)
    LAST_RESULT["exec_time_ns"] = getattr(res, "exec_time_ns", None)

    full = np.concatenate([res.results[c]["out"] for c in range(NCORES)], axis=0)
    out = np.ascontiguousarray(
        full[:G].reshape(G, L, VAL).transpose(1, 0, 2)).astype(np.float32)
    return out
